# revision 18
# baseline (speedup 1.0000x reference)
"""Bass/Trainium2 kernel for batched multi-head self-attention.

Module math (per batch b):
    q = vec @ Wq; k = vec @ Wk; v = vec @ Wv            (per head h, dim d=16)
    S = q k^T / sqrt(d);  P = softmax_j(S);  recv = P v
    out = recv @ Wo

Sharding: data-parallel over batch (8 batches -> 8 NeuronCores), weights
replicated. Each core runs an identical Bass program on its vec slice.

Precision plan (measured): fp16 anywhere in the Q/K/S path flips near-tie
softmax rows (scores have std ~128 after scaling, softmax is near one-hot)
and fails the 2e-2 gate; bf16 hi/lo pair precision suffices (4.7e-4). So
the Q/K projections, the max pass and the S'^T pass all run as float32r
matmuls (1 PE cycle/row for >=256-col streams vs 4 for plain fp32), and
only P/V/Wo run fp16. All S-path matmuls stream K=128 (zero-padded)
operands: the HAM activity monitor clock-throttles the PE to ~1.2GHz when
sustained array activity is low (an all-K=64 fp16 build ran with a ~53%
util cap for 71% of the kernel), and K-padding is free (matmul cost is
output columns, not K).

Layout: QT per round r holds heads (2r, 2r+1) at partition strips 0/64
(rows 64t+d for dim d, row 64t+16 = aug row: -rowmax, DMA'd in by the
"m-dance"); KT holds ONE head per tile, at the SAME strip as the head's
QT slot (dims at 64t+d, ones at 64t+16, rest zero). A K=128 matmul of the
QT-pair against a single-head KT selects that head via KT's zero rows, so
one QT stationary serves both heads of the pair.

Per-round pipeline:
  1. form1: S[i, j] via K=128 fp32r matmuls; DVE row-max (negated).
  2. m-dance: -rowmax -> PE transpose -> DMA into QT's aug partition.
  3. S'^T[j, i] = KT.T @ QT + ones x (-rowmax); ACT exp (scale=1/4) -> fp16
     P^T, with next round's form1 woven in between chunks.
  4. PV fp16: lhsT = [V_h | 1] (M=17) accumulates recv^T + denominator;
     both heads land in one PSUM tile (rows 0:17 / 32:49), extracted by a
     single copy + 2 strided DMAs per half.
  5. Tail: fast reciprocal + fp32r expand-matmul + fused normalize, fp16
     Wo projection.

Shapes (hardcoded): vec [8, 1024, 128]; Wq/Wk/Wv [128, 8, 16]; Wo [8, 16, 128].
vec is transposed host-side and uploaded as fp32 vecT [128, 1024].
"""

import sys

sys.path.insert(0, "/opt/trn_rl_repo")

from contextlib import ExitStack

import numpy as np

import concourse.bacc as bacc
import concourse.tile as tile
from concourse import mybir
from concourse.bass_utils import run_bass_kernel_spmd
from concourse.masks import make_identity

F32 = mybir.dt.float32
F32R = mybir.dt.float32r
F16 = mybir.dt.float16
Exp = mybir.ActivationFunctionType.Exp

B, N, X, H, D = 8, 1024, 128, 8, 16
NCHUNK = N // 128          # 8 chunks of 128 along the token dim
SCALE = 0.25               # 1/sqrt(16)
NR = 4                     # qk rounds: 2 heads each at strips {0, 64}

_CACHED_NC = None


def _r(ap):
    """View an fp32 AP as float32r for full-rate PE streaming."""
    return ap.bitcast(F32R)


def build_nc():
    """Build the per-core Bass program (identical on all cores)."""
    nc = bacc.Bacc("TRN2")

    d_wq = [nc.dram_tensor(f"wq{r}", (X, 128), F32R, kind="ExternalInput")
            for r in range(NR)]
    d_wk = [nc.dram_tensor(f"wk{h}", (X, 128), F32R, kind="ExternalInput")
            for h in range(H)]
    d_wv = nc.dram_tensor("wv", (X, 128), F32R, kind="ExternalInput")
    d_wo = nc.dram_tensor("wo", (128, X), F16, kind="ExternalInput")
    d_vecT = nc.dram_tensor("vecT", (X, N), F32R, kind="ExternalInput")
    d_e8 = nc.dram_tensor("e8c", (H, 128), F32, kind="ExternalInput")
    d_ones = nc.dram_tensor("ones", (1, N), F32R, kind="ExternalInput")
    d_out = nc.dram_tensor("out", (N, X), F32, kind="ExternalOutput")

    with tile.TileContext(nc) as tc, ExitStack() as top:
        const = top.enter_context(tc.tile_pool(name="const", bufs=1))
        ident = const.tile([128, 128], F32)
        make_identity(nc, ident)

        w_sb = {}
        for name, dram, dt in ([(f"wq{r}", d_wq[r], F32R) for r in range(NR)]
                               + [(f"wk{h}", d_wk[h], F32R) for h in range(H)]
                               + [("wv", d_wv, F32R), ("wo", d_wo, F16)]):
            t = const.tile([128, 128], dt, tag=f"w_{name}", name=f"w_{name}")
            nc.sync.dma_start(out=t[:], in_=dram[:, :])
            w_sb[name] = t

        vecT = const.tile([128, N], F32R, tag="vecT")      # [x, n]
        # split upload: projections on the first half can start while the
        # second half is still in flight.
        nc.sync.dma_start(out=vecT[:, 0:512], in_=d_vecT[:, 0:512])
        nc.sync.dma_start(out=vecT[:, 512:N], in_=d_vecT[:, 512:N])
        QT = {r: const.tile([128, N], F32R, tag=f"qt{r}", name=f"qt{r}")
              for r in range(NR)}
        KTH = {h: const.tile([128, N], F32R, tag=f"kt{h}", name=f"kt{h}")
               for h in range(H)}
        # V layout: [128 j-in-chunk, jc, 17*h + d], col 17h+16 = ones.
        V_sb = const.tile([128, NCHUNK, 17 * H], F16, tag="vsb")
        pt_pool = top.enter_context(tc.tile_pool(name="pt", bufs=3))
        raw_pool = top.enter_context(tc.tile_pool(name="raw", bufs=3))
        recvT = const.tile([128, N], F32, tag="recvT")     # [(h d), i]
        recvN = const.tile([128, N], F16, tag="recvN")     # normalized
        den_sb = const.tile([H, N], F32, tag="den")
        rden = const.tile([H, N], F32, tag="rden")
        e8 = const.tile([H, 128], F32, tag="e8")           # expand matrix
        mha_sb = const.tile([128, NCHUNK, X], F32, tag="mha")
        nc.sync.dma_start(out=e8[:], in_=d_e8[:, :])

        v_heads = V_sb[:].rearrange("p c (h s) -> p c h s", h=H)
        nc.vector.memset(v_heads[:, :, :, 16:17], 1.0)

        # ---- Phase 0: QKV projections from the uploaded vecT. ----
        with tc.tile_pool(name="ps0", bufs=2, space="PSUM") as ps0:
            # PE warmup: the Tensor engine ramps 0.65 -> 1.2 -> 2.4 GHz only
            # under continuous execution; burn idle DMA-wait time streaming
            # a memset tile so the projections start on a warm clock.
            wsrc = const.tile([128, 512], F16, tag="wsrc")
            nc.gpsimd.memset(wsrc[:], 1.0)
            warm = ps0.tile([128, 512], F32, tag="warm")
            for _ in range(8):
                nc.tensor.matmul(warm[:, :], wsrc[:, 0:128], wsrc[:, :],
                                 start=True, stop=True)
            # Q pair-projections -> QT[r]; K per-head projections -> KTH[h].
            # The PSUM matmul output rows beyond the packed head dims are
            # zeros (zero weight cols), so a full [128, N] copy zero-pads
            # the SBUF tiles for free.
            for rnd in range(NR):
                p = ps0.tile([128, N], F32, tag="proj")
                for half in range(2):
                    sl = slice(half * 512, (half + 1) * 512)
                    nc.tensor.matmul(p[:, sl], w_sb[f"wq{rnd}"][:],
                                     vecT[:, sl], start=True, stop=True)
                nc.scalar.copy(QT[rnd][:, :], p[:, :])
            for h in range(H):
                p = ps0.tile([128, N], F32, tag="proj")
                for half in range(2):
                    sl = slice(half * 512, (half + 1) * 512)
                    nc.tensor.matmul(p[:, sl], w_sb[f"wk{h}"][:],
                                     vecT[:, sl], start=True, stop=True)
                if h % 2 == 0:
                    nc.vector.tensor_copy(KTH[h][:, :], p[:, :])
                else:
                    nc.scalar.copy(KTH[h][:, :], p[:, :])
            for h in range(H):
                sp = 64 * (h % 2)
                nc.sync.dma_start(out=KTH[h][sp + 16:sp + 17, :],
                                  in_=d_ones[:, :])

            # V projection: per chunk [j, hd] = vecT[:,chunk].T @ Wv
            for c in range(NCHUNK):
                pv = ps0.tile([128, 128], F32, tag="projv")
                nc.tensor.matmul(pv[:, :], vecT[:, c * 128:(c + 1) * 128],
                                 w_sb["wv"][:], start=True, stop=True)
                dst = V_sb[:, c, :].rearrange("p (h s) -> p h s", h=H)
                src = pv[:, :].rearrange("p (h d) -> p h d", h=H)
                nc.vector.tensor_copy(dst[:, :, 0:16], src[:])

        # ---- Main loop over heads. ----
        with tc.tile_pool(name="small", bufs=6) as small, \
                tc.tile_pool(name="psm", bufs=3, space="PSUM") as psm, \
                tc.tile_pool(name="psr", bufs=2, space="PSUM") as psr:
            def emit_form1(rnd, c, m_hs):
                """One i-chunk of the fp32r max-pass for both heads of rnd.
                One QT-pair stationary serves both heads; KTH's zero rows
                select each head."""
                f1s = {}
                for h in (2 * rnd, 2 * rnd + 1):
                    f1 = psm.tile([128, N], F32, tag="big",
                                  name=f"f1_{h}_{c}")
                    f1s[h] = f1
                    for half in range(2):
                        sl = slice(half * 512, (half + 1) * 512)
                        nc.tensor.matmul(
                            f1[:, sl],
                            QT[rnd][:, c * 128:(c + 1) * 128],
                            KTH[h][:, sl], start=True, stop=True)
                for h in (2 * rnd, 2 * rnd + 1):
                    nc.vector.tensor_reduce(
                        m_hs[h][:, c:c + 1], f1s[h][:, :],
                        axis=mybir.AxisListType.X,
                        op=mybir.AluOpType.max, negate=True)

            def new_mhs(rnd):
                return {h: small.tile([128, NCHUNK], F32, tag="mh",
                                      name=f"mh{h}")
                        for h in (2 * rnd, 2 * rnd + 1)}

            # prologue: round 0 max-pass
            m_cur = new_mhs(0)
            for c in range(NCHUNK):
                emit_form1(0, c, m_cur)

            for rnd in range(NR):
                pair = (2 * rnd, 2 * rnd + 1)
                qt = QT[rnd]

                # m-dance per head: -rowmax -> aug row of QT.
                for h in pair:
                    sp = 64 * (h % 2)
                    trp = psr.tile([128, 512], F32, tag="recv",
                                   name=f"trp{h}")
                    nc.tensor.transpose(trp[0:NCHUNK, 0:128],
                                        m_cur[h][:], ident[:])
                    m8 = small.tile([NCHUNK, 128], F32R, tag="m8",
                                    name=f"m8_{h}")
                    nc.vector.tensor_copy(m8[:], trp[0:NCHUNK, 0:128])
                    nc.sync.dma_start(out=qt[sp + 16:sp + 17, :], in_=m8[:])

                # S'^T + exp, with next round's max-pass chunks woven in.
                m_nxt = new_mhs(rnd + 1) if rnd + 1 < NR else None
                PTs = {h: pt_pool.tile([128, NCHUNK * N], F16, tag="pt",
                                       name=f"pt{h}")
                       for h in pair}
                for jc in range(NCHUNK):
                    sts = {}
                    for h in pair:
                        st = psm.tile([128, N], F32, tag="big",
                                      name=f"st_{h}_{jc}")
                        sts[h] = st
                        for half in range(2):
                            sl = slice(half * 512, (half + 1) * 512)
                            nc.tensor.matmul(
                                st[:, sl],
                                KTH[h][:, jc * 128:(jc + 1) * 128],
                                qt[:, sl], start=True, stop=True)
                    for h in pair:
                        nc.scalar.activation(
                            PTs[h][:, jc * N:jc * N + N], sts[h][:, :],
                            Exp, bias=0.0, scale=SCALE)

                # PV per head with next round's form1 woven in (the S'^T
                # loop stays 2-PSUM-allocs/iter so the PE can run gap-free
                # and hold its ramped clock); extract recv + dens per half.
                for half in range(2):
                    prvs = {}
                    for hi, h in enumerate(pair):
                        prv = psr.tile([128, 512], F32, tag="recv",
                                       name=f"prv{h}_{half}")
                        prvs[h] = prv
                        for jc in range(NCHUNK):
                            nc.tensor.matmul(
                                prv[0:17, :],
                                V_sb[:, jc, 17 * h:17 * h + 17],
                                PTs[h][:, jc * N + half * 512:
                                        jc * N + (half + 1) * 512],
                                start=(jc == 0), stop=(jc == NCHUNK - 1))
                        if m_nxt is not None:
                            for c in (4 * half + 2 * hi,
                                      4 * half + 2 * hi + 1):
                                emit_form1(rnd + 1, c, m_nxt)
                    hs = slice(half * 512, (half + 1) * 512)
                    for h in pair:
                        rv = raw_pool.tile([128, 512], F32, tag="rv",
                                           name=f"rv{h}_{half}")
                        if h % 2 == 0:
                            nc.vector.tensor_copy(rv[0:17, :],
                                                  prvs[h][0:17, :])
                        else:
                            nc.scalar.copy(rv[0:17, :], prvs[h][0:17, :])
                        nc.sync.dma_start(out=recvT[16 * h:16 * h + 16, hs],
                                          in_=rv[0:16, :])
                        nc.sync.dma_start(out=den_sb[h:h + 1, hs],
                                          in_=rv[16:17, :])
                m_cur = m_nxt

        # ---- Tail: normalize + output projection. ----
        with tc.tile_pool(name="pst", bufs=2, space="PSUM") as pst, \
                tc.tile_pool(name="pstb", bufs=4, space="PSUM") as pstb:
            nc.vector.reciprocal_approx_fast(rden[:], den_sb[:])
            pe_ = pst.tile([128, N], F32, tag="expand")
            for half in range(2):
                sl = slice(half * 512, (half + 1) * 512)
                nc.tensor.matmul(pe_[:, sl], e8[:], rden[:, sl],
                                 start=True, stop=True)
            nc.vector.tensor_mul(recvN[:], recvT[:], pe_[:, :])
            for c in range(NCHUNK):
                po = pstb.tile([128, 128], F32, tag="mha")
                nc.tensor.matmul(po[:, :], recvN[:, c * 128:(c + 1) * 128],
                                 w_sb["wo"][:], start=True, stop=True)
                nc.scalar.copy(mha_sb[:, c, :], po[:, :])
                nc.sync.dma_start(out=d_out[c * 128:(c + 1) * 128, :],
                                  in_=mha_sb[:, c, :])

    nc.finalize()
    return nc


def _permute_weights(Wq, Wk, Wv, Wo):
    """Numpy-side weight layout prep."""
    def pack(W, cols):
        out = np.zeros((X, 128), dtype=np.float32)
        for base, h in cols:
            out[:, base:base + 16] = W[:, h, :]
        return out

    e8c = np.zeros((H, 128), dtype=np.float32)
    for h in range(H):
        e8c[h, 16 * h:16 * h + 16] = 1.0
    d = dict(
        wv=np.ascontiguousarray(Wv.reshape(X, 128)).astype(np.float32),
        wo=np.ascontiguousarray(Wo.reshape(128, X)).astype(np.float16),
        e8c=e8c, ones=np.ones((1, N), dtype=np.float32),
    )
    for r in range(NR):
        d[f"wq{r}"] = pack(Wq, [(0, 2 * r), (64, 2 * r + 1)])
    for h in range(H):
        d[f"wk{h}"] = pack(Wk, [(64 * (h % 2), h)])
    return d


def kernel(Wq, Wk, Wv, Wo, vec, trace=False):
    global _CACHED_NC
    if _CACHED_NC is None:
        _CACHED_NC = build_nc()
    nc = _CACHED_NC

    w = _permute_weights(np.asarray(Wq, np.float32), np.asarray(Wk, np.float32),
                         np.asarray(Wv, np.float32), np.asarray(Wo, np.float32))
    vec = np.asarray(vec, np.float32)
    in_maps = [dict(w, vecT=np.ascontiguousarray(vec[b].T)) for b in range(B)]
    res = run_bass_kernel_spmd(nc, in_maps, core_ids=list(range(B)),
                               trace=trace)
    out = np.stack([res.results[b]["out"] for b in range(B)])
    if trace:
        return out, res
    return out
